# revision 44
# baseline (speedup 1.0000x reference)
"""Causal self-attention (B=2, T=2048, C=1024, H=16, D=64) on 8 TRN2 cores.

Sharding: core c handles batch (c // 4) and the 4 heads [4*(c%4), 4*(c%4)+4).
Each core computes QKV projections for its heads, causal attention, and the
partial output projection (its heads' rows of w_proj). The host sums the 4
partials per batch (the row-sharded-matmul unshard).

On-core layout strategy (all matmuls fp32r; moving free dim >= 256 wherever
possible so fp32r streams at 1 cycle/row):
  - x^T [C, T] is supplied pre-transposed by the host, loaded as one
    [128, 8, 2048] SBUF tile in 8 chunked DMAs split across the two HWDGE
    engines (SP + Activation).
  - Q^T/K^T computed as [chan, t] tiles (chan = 2 heads x 64 stacked -> one
    128-partition tile per head pair), directly usable as S^T operands.
  - S^T[k, q] = K @ Q^T via row-tiled head pairs (two K=64 matmuls at base
    partitions 0/64 run concurrently in the PE array).
  - softmax without max-subtraction (|S|/8 <= ~6 so exp is safe in fp32):
    exp on ScalarE with the 1/sqrt(D) scale folded in; row sums come free
    via a ones column appended to V (V' = [V | 1] interleaved, stride 65).
  - causal handling: on diagonal k-tiles only the valid q range
    [128*o, 512) of the q-block is computed (o = in-block tile offset) and
    a single precomputed [128,128] triangle mask is added to the leading
    128 columns before exp.
  - S^T head slabs live at psum cols 0 and 512 (PE psum writes must not
    cross a 512-float bank boundary); one strided 2-free-dim exp covers both.
  - y'^T = V'^T @ P^T accumulated over k tiles -> [65, q] psum; row 64 is
    the softmax denominator; normalize with reciprocal + partition_broadcast
    (both exact in f32). The attention chain is software-pipelined: AV lags
    exp by `lag` tiles, and a feeder interleaves QKV/proj matmuls into the
    chain so the PE FIFO never waits on the ACT-paced exp stream.
  - proj: out[t, :] += y^T.T @ w_proj_rows, staged [128,1024] then one DMA
    per t-tile.
"""

import numpy as np

B, T, C = 2, 2048, 1024
NH, D = 16, 64
HPC = 4            # heads per core
NPAIR = 2          # head pairs per core
NCORES = 8
CK = C // 128      # 8 contraction tiles over C
TB = T // 512      # 4 q/t blocks of 512
TT = T // 128      # 16 t tiles of 128

_CACHE = {}

OPTS = {"mm": 2, "yh": 2, "pt": 5, "o3widen": False, "yc": False, "feed": 3, "lag": 3, "fmm": 2, "ob": 4, "defer_proj": "E"}


def _build(opts=None):
    opts = dict(OPTS, **(opts or {}))
    import concourse.mybir as mybir
    import concourse.tile as tile
    from concourse import bacc

    F32 = mybir.dt.float32
    F32R = mybir.dt.float32r
    EXP = mybir.ActivationFunctionType.Exp

    nc = bacc.Bacc(None, target_bir_lowering=False, debug=False)
    XT = nc.dram_tensor("xt", [C, T], F32, kind="ExternalInput")
    WQ = nc.dram_tensor("wq", [C, 256], F32, kind="ExternalInput")
    WK = nc.dram_tensor("wk", [C, 256], F32, kind="ExternalInput")
    WV = nc.dram_tensor("wv", [C, 256], F32, kind="ExternalInput")
    WP = nc.dram_tensor("wp", [256, C], F32, kind="ExternalInput")
    ONES = nc.dram_tensor("ones", [128, TT * HPC], F32, kind="ExternalInput")
    OUT = nc.dram_tensor("out", [T, C], F32, kind="ExternalOutput")

    with tile.TileContext(nc) as tc:
        with (
            tc.tile_pool(name="persist", bufs=1) as persist,
            tc.tile_pool(name="ptp", bufs=opts["pt"]) as ptp,
            tc.tile_pool(name="norm", bufs=4) as norm,
            tc.tile_pool(name="mmps", bufs=opts["mm"], space="PSUM") as mmps,
            tc.tile_pool(name="yhps", bufs=opts["yh"], space="PSUM") as yhps,
        ):
            # --- weights: one batched DMA per tensor ---
            wq_t = persist.tile([128, CK, 256], F32R, tag="wq", name="wq")
            wk_t = persist.tile([128, CK, 256], F32R, tag="wk", name="wk")
            wv_t = persist.tile([128, CK, 256], F32R, tag="wv", name="wv")
            wp_t = persist.tile([128, NPAIR, C], F32R, tag="wp", name="wp")
            xt_t = persist.tile([128, CK, T], F32R, tag="xt", name="xt")
            xt_src = XT.rearrange("(k p) t -> p k t", p=128).bitcast(F32R)
            if opts.get("batchdma", True):
                _dma_engs = [nc.sync, nc.scalar]

                def _xt_chunk(ch):
                    ksl = slice(ch, ch + 1)
                    cq = opts.get("chq", "alt")
                    eng = (_dma_engs[(ch + 1) % 2] if cq == "alt"
                           else nc.scalar)
                    eng.dma_start(out=xt_t[:, ksl, :], in_=xt_src[:, ksl, :])

                nc.sync.dma_start(
                    out=wq_t[:, :, :],
                    in_=WQ.rearrange("(k p) n -> p k n", p=128).bitcast(F32R))
                nc.sync.dma_start(
                    out=wk_t[:, :, :],
                    in_=WK.rearrange("(k p) n -> p k n", p=128).bitcast(F32R))
                for ch in range(CK):
                    _xt_chunk(ch)
                nc.sync.dma_start(
                    out=wv_t[:, :, :],
                    in_=WV.rearrange("(k p) n -> p k n", p=128).bitcast(F32R))
                nc.sync.dma_start(
                    out=wp_t[:, :, :],
                    in_=WP.rearrange("(k p) n -> p k n", p=128).bitcast(F32R))
            else:
                for k in range(CK):
                    sl = slice(k * 128, (k + 1) * 128)
                    nc.sync.dma_start(out=wq_t[:, k, :], in_=WQ[sl, :].bitcast(F32R))
                    nc.sync.dma_start(out=wk_t[:, k, :], in_=WK[sl, :].bitcast(F32R))
                    nc.sync.dma_start(out=xt_t[:, k, :], in_=XT[sl, :].bitcast(F32R))
                    nc.sync.dma_start(out=wv_t[:, k, :], in_=WV[sl, :].bitcast(F32R))
                for p in range(NPAIR):
                    nc.sync.dma_start(out=wp_t[:, p, :],
                                      in_=WP[p * 128:(p + 1) * 128, :].bitcast(F32R))

            # --- [128,128] causal triangle: 0 where q >= k else -1e9 ---
            tri = persist.tile([128, 128], F32, tag="tri", name="tri")
            nc.gpsimd.memset(tri[:, :], 0.0)
            nc.gpsimd.affine_select(
                out=tri[:, :], in_=tri[:, :],
                compare_op=mybir.AluOpType.is_ge, fill=-1e9,
                base=0, pattern=[[1, 128]], channel_multiplier=-1,
            )
            m256 = None
            if opts["o3widen"]:
                # [128,256] for o=3: cols 0:128 all -1e9, 128:256 triangle
                m256 = persist.tile([128, 256], F32, tag="m256", name="m256")
                nc.gpsimd.memset(m256[:, :], 0.0)
                nc.gpsimd.affine_select(
                    out=m256[:, :], in_=m256[:, :],
                    compare_op=mybir.AluOpType.is_ge, fill=-1e9,
                    base=-128, pattern=[[1, 256]], channel_multiplier=-1,
                )

            # --- QKV projections ---
            qt_t = [persist.tile([128, T], F32R, tag=f"qt{p}", name=f"qt{p}")
                    for p in range(NPAIR)]
            kt_t = [persist.tile([128, T], F32R, tag=f"kt{p}", name=f"kt{p}")
                    for p in range(NPAIR)]
            vp_t = persist.tile([128, TT, 260], F32R, tag="vp", name="vp")
            if opts.get("batchdma", True):
                nc.sync.dma_start(
                    out=vp_t[:, :, :].rearrange("p i (h c) -> p i h c", h=HPC)[:, :, :, 64],
                    in_=ONES.rearrange("p (i h) -> p i h", i=TT).bitcast(F32R))
            else:
                for i in range(TT):
                    nc.sync.dma_start(
                        out=vp_t[:, i, :].rearrange("p (h c) -> p h c", h=HPC)[:, :, 64],
                        in_=ONES[:, 4 * i:4 * i + 4].bitcast(F32R))
            yt_t = [persist.tile([128, T], F32R, tag=f"yt{p}", name=f"yt{p}")
                    for p in range(NPAIR)]

            # ---- feeder: QKV/proj matmul groups yielded one MM at a time so
            # they interleave into the (ACT-paced) attention chain and keep
            # the PE FIFO busy while it would otherwise wait on exp.
            def qk_group(p, tb, which):
                tsl = slice(tb * 512, (tb + 1) * 512)
                csl = slice(p * 128, (p + 1) * 128)
                dst = qt_t[p] if which == "q" else kt_t[p]
                w = wq_t if which == "q" else wk_t
                ps = mmps.tile([128, 512], F32, tag="fmm", name="psqk",
                               bufs=opts.get("fmm", 2))
                for k in range(CK):
                    nc.tensor.matmul(
                        ps[:, 0:512], w[:, k, csl], xt_t[:, k, tsl],
                        start=(k == 0), stop=(k == CK - 1))
                    yield None
                nc.vector.tensor_copy(dst[:, tsl], ps[:, 0:512])

            def v_group(i):
                tsl2 = slice(i * 128, (i + 1) * 128)
                ps = mmps.tile([128, 512], F32, tag="fmm", name="psv",
                               bufs=opts.get("fmm", 2))
                for k in range(CK):
                    nc.tensor.matmul(
                        ps[:, 0:256], xt_t[:, k, tsl2], wv_t[:, k, :],
                        start=(k == 0), stop=(k == CK - 1))
                    yield None
                nc.vector.tensor_copy(
                    vp_t[:, i, :].rearrange("p (h c) -> p h c", h=HPC)[:, :, 0:64],
                    ps[:, 0:256].rearrange("p (h c) -> p h c", h=HPC))

            def proj_group(i, late=False):
                tsl = slice(i * 128, (i + 1) * 128)
                ob = norm.tile([128, 1024], F32, tag="ob", name="ob", bufs=opts.get("ob", 2))
                for nb in range(2):
                    nsl = slice(nb * 512, (nb + 1) * 512)
                    ps = mmps.tile([128, 512], F32, tag="fmm", name="pso",
                                   bufs=opts.get("fmm", 2))
                    for p in range(NPAIR):
                        nc.tensor.matmul(
                            ps[:, 0:512], yt_t[p][:, tsl], wp_t[:, p, nsl],
                            start=(p == 0), stop=(p == NPAIR - 1))
                        yield None
                    if late and nb == 1:
                        nc.scalar.copy(ob[:, nsl], ps[:, 0:512])
                    else:
                        nc.vector.tensor_copy(ob[:, nsl], ps[:, 0:512])
                    if late:
                        (nc.sync if nb == 0 else nc.scalar).dma_start(
                            out=OUT[tsl, nsl], in_=ob[:, nsl])
                if not late:
                    nc.sync.dma_start(out=OUT[tsl, :], in_=ob[:, :])

            feeder = []   # list of active generators, consumed FIFO
            state = {"items": 0, "steps": 1}

            def feed(n):
                done = 0
                while feeder and done < n:
                    try:
                        next(feeder[0])
                        done += 1
                        state["items"] -= 1
                    except StopIteration:
                        feeder.pop(0)

            def feed_auto():
                feed(FEED)

            def feed_all():
                while feeder:
                    feed(1000)

            def push_qkv(tb, with_v=True):
                for p in range(NPAIR):
                    feeder.append(qk_group(p, tb, "q"))
                    feeder.append(qk_group(p, tb, "k"))
                    state["items"] += 2 * CK
                if with_v:
                    push_v(tb)

            def push_v(tb):
                for i in range(4 * tb, 4 * tb + 4):
                    feeder.append(v_group(i))
                    state["items"] += CK

            FEED = opts.get("feed", 4)
            LAG = opts.get("lag", 2)

            # tb0 QKV upfront (nothing to overlap with yet)
            push_qkv(0)
            if opts.get("tb1up", False):
                push_qkv(1)
                feed_all()
                state["items"] = 0
                push_qkv(2)
            else:
                feed_all()
                state["items"] = 0
                push_qkv(1)
            state["steps"] = sum((4 * qb + 4 + LAG) * NPAIR for qb in range(TB))

            for qb in range(TB):
                nkt = 4 * qb + 4
                for p in range(NPAIR):
                    pts = {}
                    qsl0 = slice(qb * 512, (qb + 1) * 512)
                    yh = [yhps.tile([65, 512], F32, tag="yh", name="yh")
                          for _ in range(2)]

                    def do_av(kt, q0, w_):
                        pt = pts.pop(kt)
                        for h in range(2):
                            vsl = slice((2 * p + h) * 65, (2 * p + h) * 65 + 65)
                            nc.tensor.matmul(
                                yh[h][:, q0:512], vp_t[:, kt, vsl],
                                pt[:, h * 512:h * 512 + w_],
                                start=(kt == 0), stop=(kt == nkt - 1),
                                skip_group_check=True)

                    geom = []
                    for kt in range(nkt):
                        o = kt - 4 * qb
                        if o > 0:
                            q0 = 256 if (o == 3 and opts["o3widen"]) else 128 * o
                        else:
                            q0 = 0
                        w_ = 512 - q0
                        geom.append((q0, w_))
                        ksl = slice(kt * 128, (kt + 1) * 128)
                        qsl = slice(qb * 512 + q0, (qb + 1) * 512)
                        # head h at col 0, head h' at col 512: PE psum
                        # writes must never cross a 512-float bank boundary
                        st = mmps.tile([128, 1024], F32, tag="mm", name="st")
                        nc.tensor.matmul(
                            st[:, 0:w_], kt_t[p][0:64, ksl], qt_t[p][0:64, qsl],
                            start=True, stop=True)
                        nc.tensor.matmul(
                            st[:, 512:512 + w_], kt_t[p][64:128, ksl],
                            qt_t[p][64:128, qsl], start=True, stop=True)
                        if o >= 0:
                            use_m256 = o == 3 and opts["o3widen"]
                            mk = m256 if use_m256 else tri
                            mw = 256 if use_m256 else 128
                            nc.vector.tensor_add(
                                st[:, 0:mw], st[:, 0:mw], mk[:, :])
                            nc.vector.tensor_add(
                                st[:, 512:512 + mw], st[:, 512:512 + mw], mk[:, :])
                        pt = ptp.tile([128, 1024], F32R, tag="pt", name="pt")
                        stv = st[:, :].rearrange("p (x c) -> p x c", x=2)
                        ptv = pt[:, :].rearrange("p (x c) -> p x c", x=2)
                        nc.scalar.activation(ptv[:, :, 0:w_], stv[:, :, 0:w_],
                                             EXP, bias=0.0, scale=0.125)
                        pts[kt] = pt
                        if kt >= LAG:
                            do_av(kt - LAG, *geom[kt - LAG])
                        feed_auto()
                    for kt in range(max(0, nkt - LAG), nkt):
                        do_av(kt, *geom[kt])
                        feed_auto()
                    # normalize: y^T[d, q] * (1 / rowsum[q])
                    for h in range(2):
                        rec = norm.tile([1, 512], F32, tag="rec", name="rec",
                                        bufs=opts.get("rb", 2))
                        nc.vector.reciprocal(rec[:, :], yh[h][64:65, :])
                        rec_b = norm.tile([64, 512], F32, tag="recb",
                                          name="recb", bufs=opts.get("rb", 2))
                        nc.gpsimd.partition_broadcast(rec_b[:, :], rec[:, :])
                        nc.vector.tensor_mul(
                            yt_t[p][h * 64:(h + 1) * 64, qsl0],
                            yh[h][0:64, :], rec_b[:, :])
                # queue this q block's projection; queue next-next QKV
                if qb == 2 and opts.get("defer_v3", True):
                    push_v(3)

                def push_proj(pqb):
                    for i in range(4 * pqb, 4 * pqb + 4):
                        feeder.append(proj_group(
                            i, late=(pqb == 3 and opts.get("late", True))))
                        state["items"] += 2 * NPAIR

                dp = opts.get("defer_proj", False)
                if dp:
                    # hold early proj work back so the starved qb3 window
                    # has feeder matmuls (schedule variants A/B/C)
                    sched = {"A": {1: [0], 2: [1, 2], 3: [3]},
                             "B": {2: [0, 1, 2], 3: [3]},
                             "C": {1: [0], 2: [1], 3: [2, 3]},
                             "D": {1: [0], 3: [1, 2, 3]},
                             "E": {2: [0, 1], 3: [2, 3]}}[
                                 dp if isinstance(dp, str) else "A"]
                    for pqb in sched.get(qb, []):
                        push_proj(pqb)
                else:
                    push_proj(qb)
                nxt = qb + 3 if opts.get("tb1up", False) else qb + 2
                if nxt <= 3:
                    push_qkv(nxt,
                             with_v=not (nxt == 3 and opts.get("defer_v3", True)))
            feed_all()

    nc.finalize()
    return nc


def _prep_in_maps(x, w_attn, w_proj):
    ones = np.ones((128, TT * HPC), dtype=np.float32)
    in_maps = []
    for c in range(NCORES):
        b, hb = c // 4, c % 4
        cols = slice(256 * hb, 256 * hb + 256)
        in_maps.append({
            "xt": np.ascontiguousarray(x[b].T),
            "wq": np.ascontiguousarray(w_attn[:, 0:C][:, cols]),
            "wk": np.ascontiguousarray(w_attn[:, C:2 * C][:, cols]),
            "wv": np.ascontiguousarray(w_attn[:, 2 * C:3 * C][:, cols]),
            "wp": np.ascontiguousarray(w_proj[256 * hb:256 * hb + 256, :]),
            "ones": ones,
        })
    return in_maps


def _run(x, w_attn, w_proj, trace=False):
    from concourse.bass_utils import run_bass_kernel_spmd

    if "nc" not in _CACHE:
        _CACHE["nc"] = _build()
    res = run_bass_kernel_spmd(
        _CACHE["nc"], _prep_in_maps(x, w_attn, w_proj),
        core_ids=list(range(NCORES)), trace=trace)
    parts = [r["out"] for r in res.results]
    out = np.empty((B, T, C), dtype=np.float32)
    for b in range(B):
        out[b] = parts[4 * b] + parts[4 * b + 1] + parts[4 * b + 2] + parts[4 * b + 3]
    return out, res


def kernel(x, w_attn, w_proj):
    out, _ = _run(np.asarray(x), np.asarray(w_attn), np.asarray(w_proj))
    return out
